# revision 13
# baseline (speedup 1.0000x reference)
"""Trainium2 Bass kernel for ConvTemporalGraphical (gnn_message_passing).

Reference computation (fp32):
    y   = einsum('nctv,oc->notv', x, W) + b        # 1x1 conv channel mix
    out = einsum('nkctv,kvw->nctw', y.reshape(n,K,C,t,v), A)

Shapes: x [16,128,256,64] f32, A [3,64,64], W [384,128], b [384].

Strategy (8 NeuronCores, data-parallel over N, 2 samples per core):
  W-contraction first, producing the intermediate TRANSPOSED so no PE
  transposes are needed anywhere:
      stage 1:  yT[(t,v), (k,c)] = sum_ci x[ci,t,v] * Wt[ci,(k,c)]
                (lhsT = x in its natural layout, fp32r, FD=384)
      stage 2:  out[c, (t,w)]   += sum_{v} yT[(t,v),(k,c)] * A[k,v,w]
                (bf16, FD=128, PSUM-accumulated over k; MA_k block-diag
                 over the two t's of a pair to use all 128 partitions)

  Engine budget: PSUM can only be drained by DVE and ACT (not GpSimd,
  not DMA), and only DVE can run TensorTensor. So both PSUM drains are
  PLAIN casts/copies, pinned to alternate between DVE and ACT, and the
  bias2[c,w] = sum_{k,v} b[(k,c)] A[k,v,w] add runs on the otherwise-
  idle GpSimd engine in SBUF (out_sb + bias -> out_sb2) before the
  output DMA. Output copies and the bias add are batched per 2 groups.

  Measured pacing (this hw): FD=384 f32r b2b 195ns/MM, FD=128 bf16 b2b
  62ns/MM -> PE ~1525ns per 8-t group = ~98us/core (the wall). DVE
  ~1280ns/group, ACT ~1125, GpSimd ~1380, SP ~613.

kernel(**inputs) shards on host, runs the SPMD program on cores 0-7, and
concatenates the per-core outputs.
"""

import numpy as np
import ml_dtypes

import concourse.bass as bass
import concourse.mybir as mybir
from concourse import bacc
from concourse.bass_utils import run_bass_kernel_spmd
from concourse.tile import TileContext

F32 = mybir.dt.float32
F32R = mybir.dt.float32r
BF16 = mybir.dt.bfloat16

N, C_IN, C_OUT, K, T, V = 16, 128, 128, 3, 256, 64
N_CORES = 8
N_PER_CORE = N // N_CORES  # 2
G = 8                      # t's per group
N_GROUPS = T // G          # 32 groups per sample


def build(reps: int = 1):
    nc = bacc.Bacc(
        "TRN2", target_bir_lowering=False, debug=False, num_devices=N_CORES
    )
    xs = nc.dram_tensor("xs", [N_PER_CORE, C_IN, T, V], F32, kind="ExternalInput")
    wt = nc.dram_tensor("wt", [C_IN, K * C_OUT], F32, kind="ExternalInput")
    mak = nc.dram_tensor("mak", [128, K, 128], BF16, kind="ExternalInput")
    bias16 = nc.dram_tensor(
        "bias16", [C_OUT, 2 * G, V], F32, kind="ExternalInput"
    )
    out = nc.dram_tensor(
        "out", [N_PER_CORE, C_OUT, T, V], F32, kind="ExternalOutput"
    )

    with TileContext(nc) as tc:
        with (
            tc.tile_pool(name="const", bufs=1) as cpool,
            tc.tile_pool(name="xin", bufs=6) as xpool,
            tc.tile_pool(name="yt", bufs=4) as ytpool,
            tc.tile_pool(name="o", bufs=5) as opool,
            tc.tile_pool(name="ps_y", bufs=2, space="PSUM") as ps_y,
            tc.tile_pool(name="ps_o", bufs=2, space="PSUM") as ps_o,
        ):
            # consts on the gpsimd DMA queue so the sync queue's first x-tile
            # descriptor issues immediately
            wt_sb = cpool.tile([C_IN, K * C_OUT], F32R, tag="wt")
            nc.gpsimd.dma_start(out=wt_sb[:], in_=wt[:].bitcast(F32R))
            mak_sb = cpool.tile([128, K, 128], BF16, tag="mak")
            nc.gpsimd.dma_start(out=mak_sb[:], in_=mak[:])
            bias_sb = cpool.tile([C_OUT, 2 * G, V], F32, tag="bias")
            nc.gpsimd.dma_start(out=bias_sb[:], in_=bias16[:])

            # Software-pipelined emission: stage 1 of group i runs while
            # stage 2 of group i-1 consumes yT drained during i's stage 1.
            for _ in range(reps):
                groups = [
                    (n, g) for n in range(N_PER_CORE) for g in range(N_GROUPS)
                ]
                st = {}

                def stage1(n, g):
                    x_sb = xpool.tile([C_IN, G * V], F32R, tag="x", name="x_sb")
                    t0 = g * G
                    nc.sync.dma_start(
                        out=x_sb[:],
                        in_=xs[n, :, t0 : t0 + G, :].bitcast(F32R),
                    )
                    yt_sb = ytpool.tile(
                        [128, 4, K * C_OUT], BF16, tag="yt", name="yt_sb"
                    )
                    # 2 pairs per 2-bank PSUM tile; one batched 768-elem
                    # drain-add per half, pinned to alternating engines
                    for h in range(2):
                        yt_ps = ps_y.tile([128, 2, 512], F32, tag="ytp")
                        for jj in range(2):
                            nc.tensor.matmul(
                                yt_ps[:, jj, 0 : K * C_OUT],
                                x_sb[:, (2 * h + jj) * 128 : (2 * h + jj + 1) * 128],
                                wt_sb[:],
                                start=True,
                                stop=True,
                            )
                        # plain drain-cast f32 -> bf16, pinned per half
                        if h == 0:
                            nc.vector.tensor_copy(
                                out=yt_sb[:, 0:2, :],
                                in_=yt_ps[:, :, 0 : K * C_OUT],
                            )
                        else:
                            nc.scalar.copy(
                                out=yt_sb[:, 2:4, :],
                                in_=yt_ps[:, :, 0 : K * C_OUT],
                            )
                    st[(n, g)] = yt_sb

                def stage2(n, g):
                    yt_sb = st.pop((n, g))
                    if g % 2 == 0:
                        st[(n, g // 2, "o2")] = ps_o.tile(
                            [C_OUT, 2, 4, 2 * V], F32, tag="op", name="o2_ps"
                        )
                    o_ps = st[(n, g // 2, "o2")]
                    for j in range(4):
                        for k in range(K):
                            nc.tensor.matmul(
                                o_ps[:, g % 2, j, :],
                                yt_sb[:, j, k * 128 : (k + 1) * 128],
                                mak_sb[:, k, :],
                                start=(k == 0),
                                stop=(k == K - 1),
                                skip_group_check=True,
                            )
                    if g % 2 == 1:
                        o_ps = st.pop((n, g // 2, "o2"))
                        # plain f32 drain of 2 groups, alternating engine
                        o_sb = opool.tile(
                            [C_OUT, 2 * G * V], F32, tag="o", name="o_sb"
                        )
                        if (g // 2) % 2 == 0:
                            nc.vector.tensor_copy(out=o_sb[:], in_=o_ps[:])
                        else:
                            nc.scalar.copy(out=o_sb[:], in_=o_ps[:])
                        # bias add on GpSimd (SBUF-only engine), then DMA
                        o_sb2 = opool.tile(
                            [C_OUT, 2 * G * V], F32, tag="o2", name="o_sb2"
                        )
                        nc.gpsimd.tensor_add(
                            out=o_sb2[:],
                            in0=o_sb[:],
                            in1=bias_sb[:],
                        )
                        t0 = (g - 1) * G
                        nc.gpsimd.dma_start(
                            out=out[n, :, t0 : t0 + 2 * G, :],
                            in_=o_sb2[:],
                        )

                for i in range(len(groups) + 2):
                    if i < len(groups):
                        stage1(*groups[i])
                    if i >= 2:
                        stage2(*groups[i - 2])

    nc.compile()
    return nc


def prep_weights(A, W, b):
    A = np.asarray(A, np.float32)
    W = np.asarray(W, np.float32)
    b = np.asarray(b, np.float32)
    # wt[ci, (k,c)]
    wt = np.ascontiguousarray(
        W.reshape(K, C_OUT, C_IN).transpose(2, 0, 1).reshape(C_IN, K * C_OUT)
    )
    # mak[(h,v), k, (h',w)] = A[k,v,w] * delta_{h,h'}
    m = np.zeros((2, V, K, 2, V), np.float32)
    for h in range(2):
        m[h, :, :, h, :] = A.transpose(1, 0, 2)
    mak = m.reshape(128, K, 128).astype(ml_dtypes.bfloat16)
    # bias2[c,w] broadcast over the 16 t's of a 2-group output tile
    bias2 = np.einsum("kc,kw->cw", b.reshape(K, C_OUT), A.sum(axis=1))
    bias16 = np.ascontiguousarray(
        np.broadcast_to(bias2[:, None, :], (C_OUT, 2 * G, V))
    ).astype(np.float32)
    return wt, mak, bias16


_NC_CACHE = {}


def get_nc(reps: int = 1):
    if reps not in _NC_CACHE:
        _NC_CACHE[reps] = build(reps)
    return _NC_CACHE[reps]


def make_in_maps(x, A, W, b):
    x = np.asarray(x, np.float32)
    wt, mak, bias16 = prep_weights(A, W, b)
    return [
        {
            "xs": np.ascontiguousarray(x[i * N_PER_CORE : (i + 1) * N_PER_CORE]),
            "wt": wt,
            "mak": mak,
            "bias16": bias16,
        }
        for i in range(N_CORES)
    ]


def run(x, A, W, b, reps: int = 1):
    nc = get_nc(reps)
    in_maps = make_in_maps(x, A, W, b)
    res = run_bass_kernel_spmd(nc, in_maps, list(range(N_CORES)))
    return np.concatenate(
        [np.asarray(res.results[i]["out"]) for i in range(N_CORES)], axis=0
    )


def kernel(x, A, W, b):
    return run(x, A, W, b, reps=1)


# revision 14
# speedup vs baseline: 1.0041x; 1.0041x over previous
"""Trainium2 Bass kernel for ConvTemporalGraphical (gnn_message_passing).

Reference computation (fp32):
    y   = einsum('nctv,oc->notv', x, W) + b        # 1x1 conv channel mix
    out = einsum('nkctv,kvw->nctw', y.reshape(n,K,C,t,v), A)

Shapes: x [16,128,256,64] f32, A [3,64,64], W [384,128], b [384].

Strategy (8 NeuronCores, data-parallel over N, 2 samples per core):
  W-contraction first, producing the intermediate TRANSPOSED so no PE
  transposes are needed anywhere:
      stage 1:  yT[(t,v), (k,c)] = sum_ci x[ci,t,v] * Wt[ci,(k,c)]
                (lhsT = x in its natural layout, fp32r, FD=384)
      stage 2:  out[c, (t,w)]   += sum_{v} yT[(t,v),(k,c)] * A[k,v,w]
                (bf16, FD=128, PSUM-accumulated over k; MA_k block-diag
                 over the two t's of a pair to use all 128 partitions)

  Engine budget: PSUM can only be drained by DVE and ACT (not GpSimd,
  not DMA), and only DVE can run TensorTensor. So both PSUM drains are
  PLAIN casts/copies, pinned to alternate between DVE and ACT, and the
  bias2[c,w] = sum_{k,v} b[(k,c)] A[k,v,w] add runs on the otherwise-
  idle GpSimd engine in SBUF (out_sb + bias -> out_sb2) before the
  output DMA. Output copies and the bias add are batched per 2 groups.

  Measured pacing (this hw): FD=384 f32r b2b 195ns/MM, FD=128 bf16 b2b
  62ns/MM -> PE ~1525ns per 8-t group = ~98us/core (the wall). DVE
  ~1280ns/group, ACT ~1125, GpSimd ~1380, SP ~613.

kernel(**inputs) shards on host, runs the SPMD program on cores 0-7, and
concatenates the per-core outputs.
"""

import numpy as np
import ml_dtypes

import concourse.bass as bass
import concourse.mybir as mybir
from concourse import bacc
from concourse.bass_utils import run_bass_kernel_spmd
from concourse.tile import TileContext

F32 = mybir.dt.float32
F32R = mybir.dt.float32r
BF16 = mybir.dt.bfloat16

N, C_IN, C_OUT, K, T, V = 16, 128, 128, 3, 256, 64
N_CORES = 8
N_PER_CORE = N // N_CORES  # 2
G = 8                      # t's per group
N_GROUPS = T // G          # 32 groups per sample


def build(reps: int = 1):
    nc = bacc.Bacc(
        "TRN2", target_bir_lowering=False, debug=False, num_devices=N_CORES
    )
    xs = nc.dram_tensor("xs", [N_PER_CORE, C_IN, T, V], F32, kind="ExternalInput")
    wt = nc.dram_tensor("wt", [C_IN, K * C_OUT], F32, kind="ExternalInput")
    mak = nc.dram_tensor("mak", [128, K, 128], BF16, kind="ExternalInput")
    bias16 = nc.dram_tensor(
        "bias16", [C_OUT, 2 * G, V], F32, kind="ExternalInput"
    )
    out = nc.dram_tensor(
        "out", [N_PER_CORE, C_OUT, T, V], F32, kind="ExternalOutput"
    )

    with TileContext(nc) as tc:
        with (
            tc.tile_pool(name="const", bufs=1) as cpool,
            tc.tile_pool(name="xin", bufs=6) as xpool,
            tc.tile_pool(name="yt", bufs=4) as ytpool,
            tc.tile_pool(name="o", bufs=5) as opool,
            tc.tile_pool(name="ps_y", bufs=2, space="PSUM") as ps_y,
            tc.tile_pool(name="ps_o", bufs=2, space="PSUM") as ps_o,
        ):
            # consts on the gpsimd DMA queue so the sync queue's first x-tile
            # descriptor issues immediately
            wt_sb = cpool.tile([C_IN, K * C_OUT], F32R, tag="wt")
            nc.gpsimd.dma_start(out=wt_sb[:], in_=wt[:].bitcast(F32R))
            mak_sb = cpool.tile([128, K, 128], BF16, tag="mak")
            nc.gpsimd.dma_start(out=mak_sb[:], in_=mak[:])
            bias_sb = cpool.tile([C_OUT, 2 * G, V], F32, tag="bias")
            nc.gpsimd.dma_start(out=bias_sb[:], in_=bias16[:])

            # Software-pipelined emission: stage 1 of group i runs while
            # stage 2 of group i-1 consumes yT drained during i's stage 1.
            for _ in range(reps):
                groups = [
                    (n, g) for n in range(N_PER_CORE) for g in range(N_GROUPS)
                ]
                st = {}

                def stage1(n, g):
                    x_sb = xpool.tile([C_IN, G * V], F32R, tag="x", name="x_sb")
                    t0 = g * G
                    nc.sync.dma_start(
                        out=x_sb[:],
                        in_=xs[n, :, t0 : t0 + G, :].bitcast(F32R),
                    )
                    yt_sb = ytpool.tile(
                        [128, 4, K * C_OUT], BF16, tag="yt", name="yt_sb"
                    )
                    # 2 pairs per 2-bank PSUM tile; one batched 768-elem
                    # drain-add per half, pinned to alternating engines
                    for h in range(2):
                        yt_ps = ps_y.tile([128, 2, 512], F32, tag="ytp")
                        for jj in range(2):
                            nc.tensor.matmul(
                                yt_ps[:, jj, 0 : K * C_OUT],
                                x_sb[:, (2 * h + jj) * 128 : (2 * h + jj + 1) * 128],
                                wt_sb[:],
                                start=True,
                                stop=True,
                            )
                        # plain drain-cast f32 -> bf16, pinned per half
                        if h == 0:
                            nc.vector.tensor_copy(
                                out=yt_sb[:, 0:2, :],
                                in_=yt_ps[:, :, 0 : K * C_OUT],
                            )
                        else:
                            nc.scalar.copy(
                                out=yt_sb[:, 2:4, :],
                                in_=yt_ps[:, :, 0 : K * C_OUT],
                            )
                    st[(n, g)] = yt_sb

                def stage2(n, g, tail):
                    yt_sb = st.pop((n, g))
                    if g % 2 == 0:
                        st[(n, g // 2, "o2")] = ps_o.tile(
                            [C_OUT, 2, 4, 2 * V], F32, tag="op", name="o2_ps"
                        )
                    o_ps = st[(n, g // 2, "o2")]
                    for j in range(4):
                        for k in range(K):
                            nc.tensor.matmul(
                                o_ps[:, g % 2, j, :],
                                yt_sb[:, j, k * 128 : (k + 1) * 128],
                                mak_sb[:, k, :],
                                start=(k == 0),
                                stop=(k == K - 1),
                                skip_group_check=True,
                            )
                    if g % 2 == 1:
                        o_ps = st.pop((n, g // 2, "o2"))
                        o_sb2 = opool.tile(
                            [C_OUT, 2 * G * V], F32, tag="o2", name="o_sb2"
                        )
                        if tail:
                            # epilogue: no yt drains left to compete with, so
                            # fuse drain+bias on DVE and skip the GpSimd hop
                            # (shortens the post-compute tail)
                            nc.vector.tensor_add(
                                out=o_sb2[:],
                                in0=o_ps[:],
                                in1=bias_sb[:],
                            )
                        else:
                            # plain f32 drain of 2 groups, alternating engine
                            o_sb = opool.tile(
                                [C_OUT, 2 * G * V], F32, tag="o", name="o_sb"
                            )
                            if (g // 2) % 2 == 0:
                                nc.vector.tensor_copy(out=o_sb[:], in_=o_ps[:])
                            else:
                                nc.scalar.copy(out=o_sb[:], in_=o_ps[:])
                            # bias add on GpSimd (SBUF-only engine)
                            nc.gpsimd.tensor_add(
                                out=o_sb2[:],
                                in0=o_sb[:],
                                in1=bias_sb[:],
                            )
                        t0 = (g - 1) * G
                        nc.gpsimd.dma_start(
                            out=out[n, :, t0 : t0 + 2 * G, :],
                            in_=o_sb2[:],
                        )

                # stage2 first: its DVE/ACT copies are ready to run, so they
                # must enqueue ahead of stage1's drains (which wait on fresh
                # matmuls) to avoid head-of-line blocking
                for i in range(len(groups) + 2):
                    if i >= 2:
                        stage2(*groups[i - 2], tail=(i - 2 >= len(groups) - 4))
                    if i < len(groups):
                        stage1(*groups[i])

    nc.compile()
    return nc


def prep_weights(A, W, b):
    A = np.asarray(A, np.float32)
    W = np.asarray(W, np.float32)
    b = np.asarray(b, np.float32)
    # wt[ci, (k,c)]
    wt = np.ascontiguousarray(
        W.reshape(K, C_OUT, C_IN).transpose(2, 0, 1).reshape(C_IN, K * C_OUT)
    )
    # mak[(h,v), k, (h',w)] = A[k,v,w] * delta_{h,h'}
    m = np.zeros((2, V, K, 2, V), np.float32)
    for h in range(2):
        m[h, :, :, h, :] = A.transpose(1, 0, 2)
    mak = m.reshape(128, K, 128).astype(ml_dtypes.bfloat16)
    # bias2[c,w] broadcast over the 16 t's of a 2-group output tile
    bias2 = np.einsum("kc,kw->cw", b.reshape(K, C_OUT), A.sum(axis=1))
    bias16 = np.ascontiguousarray(
        np.broadcast_to(bias2[:, None, :], (C_OUT, 2 * G, V))
    ).astype(np.float32)
    return wt, mak, bias16


_NC_CACHE = {}


def get_nc(reps: int = 1):
    if reps not in _NC_CACHE:
        _NC_CACHE[reps] = build(reps)
    return _NC_CACHE[reps]


def make_in_maps(x, A, W, b):
    x = np.asarray(x, np.float32)
    wt, mak, bias16 = prep_weights(A, W, b)
    return [
        {
            "xs": np.ascontiguousarray(x[i * N_PER_CORE : (i + 1) * N_PER_CORE]),
            "wt": wt,
            "mak": mak,
            "bias16": bias16,
        }
        for i in range(N_CORES)
    ]


def run(x, A, W, b, reps: int = 1):
    nc = get_nc(reps)
    in_maps = make_in_maps(x, A, W, b)
    res = run_bass_kernel_spmd(nc, in_maps, list(range(N_CORES)))
    return np.concatenate(
        [np.asarray(res.results[i]["out"]) for i in range(N_CORES)], axis=0
    )


def kernel(x, A, W, b):
    return run(x, A, W, b, reps=1)


# revision 15
# speedup vs baseline: 1.0566x; 1.0523x over previous
"""Trainium2 Bass kernel for ConvTemporalGraphical (gnn_message_passing).

Reference computation (fp32):
    y   = einsum('nctv,oc->notv', x, W) + b        # 1x1 conv channel mix
    out = einsum('nkctv,kvw->nctw', y.reshape(n,K,C,t,v), A)

Shapes: x [16,128,256,64] f32, A [3,64,64], W [384,128], b [384].

Strategy (8 NeuronCores, data-parallel over N, 2 samples per core):
  W-contraction first, producing the intermediate TRANSPOSED so no PE
  transposes are needed anywhere:
      stage 1:  yT[(t,v), (k,c)] = sum_ci x[ci,t,v] * Wt[ci,(k,c)]
                (lhsT = x in its natural layout, fp32r, FD=384)
      stage 2:  out[c, (t,w)]   += sum_{v} yT[(t,v),(k,c)] * A[k,v,w]
                (bf16, FD=128, PSUM-accumulated over k; MA_k block-diag
                 over the two t's of a pair to use all 128 partitions)

  Engine budget: PSUM can only be drained by DVE and ACT (not GpSimd,
  not DMA), and only DVE can run TensorTensor. So both PSUM drains are
  PLAIN casts/copies, pinned to alternate between DVE and ACT, and the
  bias2[c,w] = sum_{k,v} b[(k,c)] A[k,v,w] add runs on the otherwise-
  idle GpSimd engine in SBUF (out_sb + bias -> out_sb2) before the
  output DMA. Output copies and the bias add are batched per 2 groups.

  Measured pacing (this hw): FD=384 f32r b2b 195ns/MM, FD=128 bf16 b2b
  62ns/MM -> PE ~1525ns per 8-t group = ~98us/core (the wall). DVE
  ~1280ns/group, ACT ~1125, GpSimd ~1380, SP ~613.

kernel(**inputs) shards on host, runs the SPMD program on cores 0-7, and
concatenates the per-core outputs.
"""

import numpy as np
import ml_dtypes

import concourse.bass as bass
import concourse.mybir as mybir
from concourse import bacc
from concourse.bass_utils import run_bass_kernel_spmd
from concourse.tile import TileContext

F32 = mybir.dt.float32
F32R = mybir.dt.float32r
BF16 = mybir.dt.bfloat16

N, C_IN, C_OUT, K, T, V = 16, 128, 128, 3, 256, 64
N_CORES = 8
N_PER_CORE = N // N_CORES  # 2
G = 8                      # t's per group
N_GROUPS = T // G          # 32 groups per sample


def build(reps: int = 1):
    nc = bacc.Bacc(
        "TRN2", target_bir_lowering=False, debug=False, num_devices=N_CORES
    )
    xs = nc.dram_tensor("xs", [N_PER_CORE, C_IN, T, V], F32, kind="ExternalInput")
    wt = nc.dram_tensor("wt", [C_IN, K * C_OUT], F32, kind="ExternalInput")
    mak = nc.dram_tensor("mak", [128, K, 128], BF16, kind="ExternalInput")
    bias8 = nc.dram_tensor(
        "bias8", [C_OUT, G, V], F32, kind="ExternalInput"
    )
    out = nc.dram_tensor(
        "out", [N_PER_CORE, C_OUT, T, V], F32, kind="ExternalOutput"
    )

    with TileContext(nc) as tc:
        with (
            tc.tile_pool(name="const", bufs=1) as cpool,
            tc.tile_pool(name="xin", bufs=6) as xpool,
            tc.tile_pool(name="yt", bufs=4) as ytpool,
            tc.tile_pool(name="o", bufs=5) as opool,
            tc.tile_pool(name="ps_y", bufs=2, space="PSUM") as ps_y,
            tc.tile_pool(name="ps_o", bufs=4, space="PSUM") as ps_o,
        ):
            # consts on the gpsimd DMA queue so the sync queue's first x-tile
            # descriptor issues immediately
            wt_sb = cpool.tile([C_IN, K * C_OUT], F32R, tag="wt")
            nc.gpsimd.dma_start(out=wt_sb[:], in_=wt[:].bitcast(F32R))
            mak_sb = cpool.tile([128, K, 128], BF16, tag="mak")
            nc.gpsimd.dma_start(out=mak_sb[:], in_=mak[:])
            bias_sb = cpool.tile([C_OUT, G, V], F32, tag="bias")
            nc.gpsimd.dma_start(out=bias_sb[:], in_=bias8[:])

            # Software-pipelined emission: stage 1 of group i runs while
            # stage 2 of group i-1 consumes yT drained during i's stage 1.
            for _ in range(reps):
                groups = [
                    (n, g) for n in range(N_PER_CORE) for g in range(N_GROUPS)
                ]
                st = {}

                def stage1(n, g):
                    x_sb = xpool.tile([C_IN, G * V], F32R, tag="x", name="x_sb")
                    t0 = g * G
                    nc.sync.dma_start(
                        out=x_sb[:],
                        in_=xs[n, :, t0 : t0 + G, :].bitcast(F32R),
                    )
                    yt_sb = ytpool.tile(
                        [128, 4, K * C_OUT], BF16, tag="yt", name="yt_sb"
                    )
                    # 2 pairs per 2-bank PSUM tile; one batched 768-elem
                    # drain-add per half, pinned to alternating engines
                    for h in range(2):
                        yt_ps = ps_y.tile([128, 2, 512], F32, tag="ytp")
                        for jj in range(2):
                            nc.tensor.matmul(
                                yt_ps[:, jj, 0 : K * C_OUT],
                                x_sb[:, (2 * h + jj) * 128 : (2 * h + jj + 1) * 128],
                                wt_sb[:],
                                start=True,
                                stop=True,
                            )
                        # plain drain-cast f32 -> bf16, pinned per half
                        if h == 0:
                            nc.vector.tensor_copy(
                                out=yt_sb[:, 0:2, :],
                                in_=yt_ps[:, :, 0 : K * C_OUT],
                            )
                        else:
                            nc.scalar.copy(
                                out=yt_sb[:, 2:4, :],
                                in_=yt_ps[:, :, 0 : K * C_OUT],
                            )
                    st[(n, g)] = yt_sb

                def stage2(n, g, tail):
                    yt_sb = st.pop((n, g))
                    o_ps = ps_o.tile(
                        [C_OUT, 4, 2 * V], F32, tag="op", name="o2_ps"
                    )
                    for j in range(4):
                        for k in range(K):
                            nc.tensor.matmul(
                                o_ps[:, j, :],
                                yt_sb[:, j, k * 128 : (k + 1) * 128],
                                mak_sb[:, k, :],
                                start=(k == 0),
                                stop=(k == K - 1),
                                skip_group_check=True,
                            )
                    o_sb2 = opool.tile(
                        [C_OUT, G * V], F32, tag="o2", name="o_sb2"
                    )
                    if tail:
                        # epilogue: no yt drains left to compete with, so
                        # fuse drain+bias on DVE and skip the GpSimd hop
                        # (shortens the post-compute tail)
                        nc.vector.tensor_add(
                            out=o_sb2[:],
                            in0=o_ps[:],
                            in1=bias_sb[:],
                        )
                    else:
                        # plain f32 drain, alternating engine; bias on the
                        # GpSimd engine (SBUF-only)
                        o_sb = opool.tile(
                            [C_OUT, G * V], F32, tag="o", name="o_sb"
                        )
                        if g % 2 == 0:
                            nc.vector.tensor_copy(out=o_sb[:], in_=o_ps[:])
                        else:
                            nc.scalar.copy(out=o_sb[:], in_=o_ps[:])
                        nc.gpsimd.tensor_add(
                            out=o_sb2[:],
                            in0=o_sb[:],
                            in1=bias_sb[:],
                        )
                    # out-DMA descgen on the SP queue (GpSimd is TT-bound)
                    t0 = g * G
                    nc.sync.dma_start(
                        out=out[n, :, t0 : t0 + G, :],
                        in_=o_sb2[:],
                    )

                # stage2 first: its DVE/ACT copies are ready to run, so they
                # must enqueue ahead of stage1's drains (which wait on fresh
                # matmuls) to avoid head-of-line blocking
                for i in range(len(groups) + 2):
                    if i >= 2:
                        stage2(*groups[i - 2], tail=(i - 2 >= len(groups) - 4))
                    if i < len(groups):
                        stage1(*groups[i])

    nc.compile()
    return nc


def prep_weights(A, W, b):
    A = np.asarray(A, np.float32)
    W = np.asarray(W, np.float32)
    b = np.asarray(b, np.float32)
    # wt[ci, (k,c)]
    wt = np.ascontiguousarray(
        W.reshape(K, C_OUT, C_IN).transpose(2, 0, 1).reshape(C_IN, K * C_OUT)
    )
    # mak[(h,v), k, (h',w)] = A[k,v,w] * delta_{h,h'}
    m = np.zeros((2, V, K, 2, V), np.float32)
    for h in range(2):
        m[h, :, :, h, :] = A.transpose(1, 0, 2)
    mak = m.reshape(128, K, 128).astype(ml_dtypes.bfloat16)
    # bias2[c,w] broadcast over the 8 t's of a group output tile
    bias2 = np.einsum("kc,kw->cw", b.reshape(K, C_OUT), A.sum(axis=1))
    bias8 = np.ascontiguousarray(
        np.broadcast_to(bias2[:, None, :], (C_OUT, G, V))
    ).astype(np.float32)
    return wt, mak, bias8


_NC_CACHE = {}


def get_nc(reps: int = 1):
    if reps not in _NC_CACHE:
        _NC_CACHE[reps] = build(reps)
    return _NC_CACHE[reps]


def make_in_maps(x, A, W, b):
    x = np.asarray(x, np.float32)
    wt, mak, bias8 = prep_weights(A, W, b)
    return [
        {
            "xs": np.ascontiguousarray(x[i * N_PER_CORE : (i + 1) * N_PER_CORE]),
            "wt": wt,
            "mak": mak,
            "bias8": bias8,
        }
        for i in range(N_CORES)
    ]


def run(x, A, W, b, reps: int = 1):
    nc = get_nc(reps)
    in_maps = make_in_maps(x, A, W, b)
    res = run_bass_kernel_spmd(nc, in_maps, list(range(N_CORES)))
    return np.concatenate(
        [np.asarray(res.results[i]["out"]) for i in range(N_CORES)], axis=0
    )


def kernel(x, A, W, b):
    return run(x, A, W, b, reps=1)
